# revision 3
# baseline (speedup 1.0000x reference)
"""Bass program builder for the BVH skeleton forward-kinematics kernel.

Per-core workload: 8192 frames laid out as [128 partitions x 64 frames].
Quaternion-based FK:
  1. to-quat: axis-angle poses -> local unit quaternions (ACT + DVE),
     split into joint groups so the tree compose can start early
  2. compose: cumulative quats down the tree, batched per level (DVE)
  3. rotate:  V_c = V_p + w*t + qv x t, t = qv x (2*o_c)  (DVE, coef tiles)
  4. out:     positions = V + base (telescoped constant offsets), written in
     (quantity, frame) layout; the host transposes to [B, J, 3]

Frame b in the core shard maps to (partition p = b // 64, slot f = b % 64).
"""

import numpy as np

import concourse.bass as bass
import concourse.tile as tile
from concourse import bacc, mybir

F = 64          # frames per partition
P = 128         # partitions
BC = P * F      # frames per core = 8192
J = 24
NJQ = 22        # joints 0..21 get local quats (22,23 are trailing leaves)

PARENTS = [-1, 0, 0, 0, 1, 2, 3, 4, 5, 6, 7, 8, 9, 9, 9, 12, 13, 14, 16, 17, 18, 19, 20, 21]

# (child_lo, child_hi, parent_lo, parent_stride, parent_is_local)
COMPOSE_LEVELS = [
    (1, 4, 0, 0, True),
    (4, 7, 1, 1, False),
    (7, 10, 4, 1, False),
    (12, 15, 9, 0, False),
    (16, 18, 13, 1, False),
    (18, 20, 16, 1, False),
    (20, 22, 18, 1, False),
]
ROTATE_LEVELS = [
    (1, 4, 0, 0, True),
    (4, 7, 1, 1, False),
    (7, 10, 4, 1, False),
    (10, 13, 7, 1, False),
    (13, 15, 9, 0, False),
    (15, 18, 12, 1, False),
    (18, 20, 16, 1, False),
    (20, 22, 18, 1, False),
    (22, 24, 20, 1, False),
]

# stage-1 joint groups: group 0 unblocks compose L1 early
JGROUPS = [(0, 4), (4, 10), (10, 22)]

# output DMA chunks: (joint_lo, joint_hi, after_rotate_level_index)
OUT_CHUNKS = [(0, 10, 2), (10, 18, 5), (18, 24, 8)]

EPS = 1e-8
HALF_PI = float(np.pi / 2)

CO1_BASE = 0
CO2_BASE = 69
BASE_BASE = 138
EPS_COL = 210
HPI_COL = 211
PI_COL = 212
NCOEF = 213

DUPW = 5  # dup-vector slots per joint: [x, y, z, x, y]


def make_coef(offsets: np.ndarray) -> np.ndarray:
    offsets = np.asarray(offsets, dtype=np.float32)
    row = np.zeros((NCOEF,), dtype=np.float32)
    for c in range(1, 24):
        for i in range(3):
            row[CO1_BASE + (c - 1) * 3 + i] = 2.0 * offsets[c][(i + 2) % 3]
            row[CO2_BASE + (c - 1) * 3 + i] = 2.0 * offsets[c][(i + 1) % 3]
    base = np.zeros((24, 3), dtype=np.float32)
    base[0] = offsets[0]
    for j in range(1, 24):
        base[j] = base[PARENTS[j]] + offsets[j]
    row[BASE_BASE:BASE_BASE + 72] = base.reshape(-1)
    row[EPS_COL] = EPS
    row[HPI_COL] = HALF_PI
    row[PI_COL] = float(np.pi)
    return np.ascontiguousarray(np.broadcast_to(row, (P, NCOEF)))


def _ap(t_ap: bass.AP, off: int, dims) -> bass.AP:
    th = t_ap.tensor
    n = th.shape[1]
    return bass.AP(th, off, [[n, P]] + [[int(s), int(c)] for (s, c) in dims])


class Cfg:
    dt_q = mybir.dt.float32    # quaternion chain dtype
    dt_v = mybir.dt.float32    # V accumulation dtype
    coef16 = False             # keep a dt_q copy of COEF for 2x-mode muls
    final_on_gpsimd = False    # base-adds for joints < 18 on GPSIMD
    s1_on_gpsimd = False       # stage-1 k2/qv for groups 1,2 on GPSIMD
    dma_split = 2              # split input poses DMA along f
    split_out = True           # chunked output DMAs in (q, f) layout


def build_fk(tc: tile.TileContext, cfg: Cfg = None):
    cfg = cfg or Cfg()
    nc = tc.nc
    f32 = mybir.dt.float32
    dt_q, dt_v = cfg.dt_q, cfg.dt_v
    A = mybir.ActivationFunctionType
    OP = mybir.AluOpType

    poses = nc.dram_tensor("poses", [BC, J * 3], f32, kind="ExternalInput")
    trans = nc.dram_tensor("trans", [BC, 3], f32, kind="ExternalInput")
    coefd = nc.dram_tensor("coef", [P, NCOEF], f32, kind="ExternalInput")
    outd = nc.dram_tensor("positions", [P, 72 * F], f32, kind="ExternalOutput")

    pool = tc.alloc_tile_pool(name="main", bufs=1)

    def mk(name, n, dt):
        return pool.tile([P, n], dt, name=name)

    PR = mk("PR", F * 72, f32)
    TR = mk("TR", F * 3, f32)
    COEF = mk("COEF", NCOEF, f32)
    C16 = mk("C16", NCOEF, dt_q) if cfg.coef16 else None
    SQ = mk("SQ", 66 * F, f32)
    N2 = mk("N2", NJQ * F, f32)
    ANG = mk("ANG", NJQ * F, f32)
    INV = mk("INV", NJQ * F, f32)
    SH = mk("SH", NJQ * F, f32)
    K2 = mk("K2", NJQ * F, f32)
    LW = mk("LW", NJQ * F, dt_q)
    LD = mk("LD", NJQ * DUPW * F, dt_q)
    QW = mk("QW", NJQ * F, dt_q)
    QD = mk("QD", NJQ * DUPW * F, dt_q)
    V = mk("V", J * 3 * F, dt_v)
    OUT = mk("OUT", 72 * F, f32)          # dense (q, f): q*F + f

    T1 = mk("T1", 3 * 3 * F, dt_q)
    T2 = mk("T2", 3 * 3 * F, dt_q)
    T3 = mk("T3", 3 * 3 * F, dt_q)
    T4 = mk("T4", 3 * 3 * F, dt_q)
    V1 = mk("V1", 3 * 3 * F, dt_q)
    V2 = mk("V2", 3 * 3 * F, dt_q)
    PD = mk("PD", 3 * 3 * F, dt_q)
    PW = mk("PW", 3 * F, dt_q)
    TD = mk("TD", 3 * DUPW * F, dt_q)
    ZS = mk("ZS", 3 * 3 * F, dt_v)

    COEF_M = C16 if cfg.coef16 else COEF  # coef source for the 2x-mode muls

    # ---- DMA in: COEF/TR first (tiny, unblock ACT bias), poses split along
    # f across two DMA queues ----
    ns = cfg.dma_split
    fchunk = F // ns
    nc.sync.dma_start(COEF[:], coefd.ap())
    nc.sync.dma_start(TR[:], bass.AP(trans, 0, [[F * 3, P], [1, F * 3]]))
    dma_engs = [nc.sync, nc.scalar]
    for s in range(ns):
        dma_engs[s % len(dma_engs)].dma_start(
            _ap(PR, s * fchunk * 72, [(1, fchunk * 72)]),
            bass.AP(poses, s * fchunk * 72, [[F * 72, P], [1, fchunk * 72]]),
        )
    if cfg.coef16:
        nc.vector.tensor_copy(C16[:], COEF[:])

    # ---- stage 1: to-quat, per joint group x f-chunk ----
    def stage1(gi, jlo, jhi):
        njg = jhi - jlo
        # phase A (sqrt table set)
        for s in range(ns):
            f0, nf = s * fchunk, fchunk
            nc.scalar.activation(
                _ap(SQ, jlo * 3 * F + f0, [(F, 3 * njg), (1, nf)]),
                _ap(PR, f0 * 72 + jlo * 3, [(1, 3 * njg), (72, nf)]),
                A.Square, bias=_ap(COEF, EPS_COL, [(1, 1)]), scale=1.0,
            )
            nc.vector.tensor_tensor(
                _ap(N2, jlo * F + f0, [(F, njg), (1, nf)]),
                _ap(SQ, jlo * 3 * F + f0, [(3 * F, njg), (1, nf)]),
                _ap(SQ, jlo * 3 * F + F + f0, [(3 * F, njg), (1, nf)]),
                OP.add,
            )
            nc.vector.tensor_tensor(
                _ap(N2, jlo * F + f0, [(F, njg), (1, nf)]),
                _ap(N2, jlo * F + f0, [(F, njg), (1, nf)]),
                _ap(SQ, jlo * 3 * F + 2 * F + f0, [(3 * F, njg), (1, nf)]),
                OP.add,
            )
            nc.scalar.activation(
                _ap(ANG, jlo * F + f0, [(F, njg), (1, nf)]),
                _ap(N2, jlo * F + f0, [(F, njg), (1, nf)]), A.Sqrt)
            nc.vector.reciprocal_approx_fast(
                _ap(INV, jlo * F + f0, [(F, njg), (1, nf)]),
                _ap(ANG, jlo * F + f0, [(F, njg), (1, nf)]))
        # phase B (trig table set): sh = sin(pi - a/2); cos = sin(pi/2 - a/2)
        eng = nc.gpsimd if (cfg.s1_on_gpsimd and gi == 2) else nc.vector
        for s in range(ns):
            f0, nf = s * fchunk, fchunk
            nc.scalar.activation(
                _ap(SH, jlo * F + f0, [(F, njg), (1, nf)]),
                _ap(ANG, jlo * F + f0, [(F, njg), (1, nf)]), A.Sin, scale=-0.5,
                bias=_ap(COEF, PI_COL, [(1, 1)]))
            nc.scalar.activation(
                _ap(LW, jlo * F + f0, [(F, njg), (1, nf)]),
                _ap(ANG, jlo * F + f0, [(F, njg), (1, nf)]), A.Sin, scale=-0.5,
                bias=_ap(COEF, HPI_COL, [(1, 1)]),
            )
            eng.tensor_tensor(
                _ap(K2, jlo * F + f0, [(F, njg), (1, nf)]),
                _ap(SH, jlo * F + f0, [(F, njg), (1, nf)]),
                _ap(INV, jlo * F + f0, [(F, njg), (1, nf)]), OP.mult)
            eng.tensor_tensor(
                _ap(LD, jlo * DUPW * F + f0, [(DUPW * F, njg), (F, 3), (1, nf)]),
                _ap(K2, jlo * F + f0, [(F, njg), (0, 3), (1, nf)]),
                _ap(PR, f0 * 72 + jlo * 3, [(3, njg), (1, 3), (72, nf)]),
                OP.mult,
            )
        eng.tensor_copy(
            _ap(LD, jlo * DUPW * F + 3 * F, [(DUPW * F, njg), (1, 2 * F)]),
            _ap(LD, jlo * DUPW * F, [(DUPW * F, njg), (1, 2 * F)]),
        )

    stage1(0, *JGROUPS[0])
    # V[0] = TR rearranged (i, f)
    nc.vector.tensor_copy(
        _ap(V, 0, [(F, 3), (1, F)]),
        _ap(TR, 0, [(1, 3), (3, F)]),
    )
    stage1(1, *JGROUPS[1])
    stage1(2, *JGROUPS[2])

    def emit_final_add(jlo, jhi, eng):
        # OUT[3j+i, f] = V[3j+i, f] + base[3j+i]  (dense q-major)
        q0, nq = jlo * 3, (jhi - jlo) * 3
        eng.tensor_tensor(
            _ap(OUT, q0 * F, [(F, nq), (1, F)]),
            _ap(V, q0 * F, [(F, nq), (1, F)]),
            _ap(COEF, BASE_BASE + q0, [(1, nq), (0, F)]),
            OP.add,
        )

    fin_eng0 = nc.gpsimd if cfg.final_on_gpsimd else nc.vector
    emit_final_add(0, 1, fin_eng0)

    def qsrc(local):
        return (LW, LD) if local else (QW, QD)

    # ---- stage 2 + 3 interleaved per level would be natural, but compose
    # levels run ahead of rotate anyway; emit compose then rotate.
    for (clo, chi, plo, pstr, plocal) in COMPOSE_LEVELS:
        b = chi - clo
        w_, d_ = qsrc(plocal)
        pw = _ap(w_, plo * F, [(pstr * F, b), (0, 3), (1, F)])
        pwf = _ap(w_, plo * F, [(pstr * F, b), (1, F)])
        pv = _ap(d_, plo * DUPW * F, [(pstr * DUPW * F, b), (F, 3), (1, F)])
        pv1 = _ap(d_, plo * DUPW * F + F, [(pstr * DUPW * F, b), (F, 3), (1, F)])
        pv2 = _ap(d_, plo * DUPW * F + 2 * F, [(pstr * DUPW * F, b), (F, 3), (1, F)])
        cw = _ap(LW, clo * F, [(F, b), (0, 3), (1, F)])
        cwf = _ap(LW, clo * F, [(F, b), (1, F)])
        cv = _ap(LD, clo * DUPW * F, [(DUPW * F, b), (F, 3), (1, F)])
        cv1 = _ap(LD, clo * DUPW * F + F, [(DUPW * F, b), (F, 3), (1, F)])
        cv2 = _ap(LD, clo * DUPW * F + 2 * F, [(DUPW * F, b), (F, 3), (1, F)])

        def s3(t):
            return _ap(t, 0, [(3 * F, b), (F, 3), (1, F)])

        t1, t2, t3, t4, v1, v2, pd = map(s3, (T1, T2, T3, T4, V1, V2, PD))

        nc.vector.tensor_tensor(t1, pw, cv, OP.mult)
        nc.vector.tensor_tensor(t2, pv, cw, OP.mult)
        nc.vector.tensor_tensor(t3, pv1, cv2, OP.mult)
        nc.vector.tensor_tensor(t4, pv2, cv1, OP.mult)
        nc.vector.tensor_tensor(v1, t1, t2, OP.add)
        nc.vector.tensor_tensor(v2, t3, t4, OP.subtract)
        qdv = _ap(QD, clo * DUPW * F, [(DUPW * F, b), (F, 3), (1, F)])
        nc.vector.tensor_tensor(qdv, v1, v2, OP.add)
        nc.vector.tensor_copy(
            _ap(QD, clo * DUPW * F + 3 * F, [(DUPW * F, b), (1, 2 * F)]),
            _ap(QD, clo * DUPW * F, [(DUPW * F, b), (1, 2 * F)]),
        )
        nc.vector.tensor_tensor(pd, pv, cv, OP.mult)
        pwc = _ap(PW, 0, [(F, b), (1, F)])
        nc.vector.tensor_tensor(pwc, pwf, cwf, OP.mult)
        qwc = _ap(QW, clo * F, [(F, b), (1, F)])
        nc.vector.tensor_tensor(qwc, pwc, _ap(PD, 0, [(3 * F, b), (1, F)]), OP.subtract)
        nc.vector.tensor_tensor(qwc, qwc, _ap(PD, F, [(3 * F, b), (1, F)]), OP.subtract)
        nc.vector.tensor_tensor(qwc, qwc, _ap(PD, 2 * F, [(3 * F, b), (1, F)]), OP.subtract)

    # ---- stage 3: rotate offsets, accumulate V; final adds + out DMA chunks ----
    chunk_after = {lvl: (jlo, jhi) for (jlo, jhi, lvl) in OUT_CHUNKS}
    for li, (clo, chi, plo, pstr, plocal) in enumerate(ROTATE_LEVELS):
        b = chi - clo
        w_, d_ = qsrc(plocal)
        pwk = _ap(w_, plo * F, [(pstr * F, b), (0, 3), (1, F)])
        pv1 = _ap(d_, plo * DUPW * F + F, [(pstr * DUPW * F, b), (F, 3), (1, F)])
        pv2 = _ap(d_, plo * DUPW * F + 2 * F, [(pstr * DUPW * F, b), (F, 3), (1, F)])
        co1 = _ap(COEF_M, CO1_BASE + (clo - 1) * 3, [(3, b), (1, 3), (0, F)])
        co2 = _ap(COEF_M, CO2_BASE + (clo - 1) * 3, [(3, b), (1, 3), (0, F)])

        def s3(t):
            return _ap(t, 0, [(3 * F, b), (F, 3), (1, F)])

        r1, r2, m1, m2, u, sv = map(s3, (T1, T2, T3, T4, V1, V2))
        z = _ap(ZS, 0, [(3 * F, b), (F, 3), (1, F)])

        nc.vector.tensor_tensor(r1, pv1, co1, OP.mult)
        nc.vector.tensor_tensor(r2, pv2, co2, OP.mult)
        td0 = _ap(TD, 0, [(DUPW * F, b), (F, 3), (1, F)])
        nc.vector.tensor_tensor(td0, r1, r2, OP.subtract)
        nc.vector.tensor_copy(
            _ap(TD, 3 * F, [(DUPW * F, b), (1, 2 * F)]),
            _ap(TD, 0, [(DUPW * F, b), (1, 2 * F)]),
        )
        td1 = _ap(TD, F, [(DUPW * F, b), (F, 3), (1, F)])
        td2 = _ap(TD, 2 * F, [(DUPW * F, b), (F, 3), (1, F)])
        nc.vector.tensor_tensor(m1, pv1, td2, OP.mult)
        nc.vector.tensor_tensor(m2, pv2, td1, OP.mult)
        nc.vector.tensor_tensor(u, m1, m2, OP.subtract)
        nc.vector.tensor_tensor(sv, pwk, td0, OP.mult)
        nc.vector.tensor_tensor(z, sv, u, OP.add)
        vc = _ap(V, clo * 3 * F, [(3 * F, b), (F, 3), (1, F)])
        vp = _ap(V, plo * 3 * F, [(pstr * 3 * F, b), (F, 3), (1, F)])
        nc.vector.tensor_tensor(vc, vp, z, OP.add)
        fin_eng = nc.gpsimd if (cfg.final_on_gpsimd and clo < 18) else nc.vector
        emit_final_add(clo, chi, fin_eng)
        if cfg.split_out and li in chunk_after:
            jlo, jhi = chunk_after[li]
            q0, nq = jlo * 3 * F, (jhi - jlo) * 3 * F
            nc.sync.dma_start(
                bass.AP(outd, q0, [[72 * F, P], [1, nq]]),
                _ap(OUT, q0, [(1, nq)]),
            )
    if not cfg.split_out:
        nc.sync.dma_start(bass.AP(outd, 0, [[72 * F, P], [1, 72 * F]]), OUT[:])
    pool.release()


def build_program(dt_q=mybir.dt.float32, trn="TRN2", cfg: Cfg = None):
    if cfg is None:
        cfg = Cfg()
        cfg.dt_q = dt_q
    nc = bacc.Bacc(trn, target_bir_lowering=False, debug=False)
    with tile.TileContext(nc) as tc:
        build_fk(tc, cfg)
    nc.compile()
    return nc


def shard_inputs(inputs: dict, n_cores: int = 8):
    poses = np.asarray(inputs["poses"], dtype=np.float32).reshape(-1, J * 3)
    trans = np.asarray(inputs["trans"], dtype=np.float32).reshape(-1, 3)
    coef = make_coef(np.asarray(inputs["offsets"], dtype=np.float32))
    bc = poses.shape[0] // n_cores
    in_maps = []
    for c in range(n_cores):
        in_maps.append({
            "poses": np.ascontiguousarray(poses[c * bc:(c + 1) * bc]),
            "trans": np.ascontiguousarray(trans[c * bc:(c + 1) * bc]),
            "coef": coef,
        })
    return in_maps


def unshard_outputs(results) -> np.ndarray:
    # per-core output is [P, 72*F] in (partition, q, f) layout
    outs = []
    for r in results:
        o = r["positions"].reshape(P, 72, F).transpose(0, 2, 1)  # -> (p, f, q)
        outs.append(o.reshape(BC, J, 3))
    return np.concatenate(outs, axis=0)


# ======================== runtime entry point ========================

from concourse import bass_utils  # noqa: E402

N_CORES = 8
B = BC * N_CORES

LAST_EXEC_NS = None
_CACHED = {}


def _default_cfg():
    cfg = Cfg()
    cfg.dt_q = mybir.dt.float16
    cfg.dt_v = mybir.dt.float16
    cfg.coef16 = True
    cfg.final_on_gpsimd = True
    cfg.s1_on_gpsimd = True
    cfg.dma_split = 2
    cfg.split_out = True
    return cfg


def _get_program():
    if "nc" not in _CACHED:
        _CACHED["nc"] = build_program(cfg=_default_cfg())
    return _CACHED["nc"]


def kernel(offsets: np.ndarray, poses: np.ndarray, trans: np.ndarray) -> np.ndarray:
    global LAST_EXEC_NS
    nc = _get_program()
    in_maps = shard_inputs(
        {"offsets": offsets, "poses": poses, "trans": trans}, n_cores=N_CORES
    )
    res = bass_utils.run_bass_kernel_spmd(
        nc, in_maps, core_ids=list(range(N_CORES)),
    )
    LAST_EXEC_NS = res.exec_time_ns
    out = unshard_outputs(res.results)
    return np.ascontiguousarray(out.astype(np.float32))


# revision 4
# speedup vs baseline: 14450.9752x; 14450.9752x over previous
"""Bass program builder for the BVH skeleton forward-kinematics kernel.

Per-core workload: 8192 frames laid out as [128 partitions x 64 frames].
Quaternion-based FK:
  1. to-quat: axis-angle poses -> local unit quaternions (ACT + DVE),
     split into joint groups so the tree compose can start early
  2. compose: cumulative quats down the tree, batched per level (DVE)
  3. rotate:  V_c = V_p + w*t + qv x t, t = qv x (2*o_c)  (DVE, coef tiles)
  4. out:     positions = V + base (telescoped constant offsets), written in
     (quantity, frame) layout; the host transposes to [B, J, 3]

Frame b in the core shard maps to (partition p = b // 64, slot f = b % 64).
"""

import numpy as np

import concourse.bass as bass
import concourse.tile as tile
from concourse import bacc, mybir

F = 64          # frames per partition
P = 128         # partitions
BC = P * F      # frames per core = 8192
J = 24
NJQ = 22        # joints 0..21 get local quats (22,23 are trailing leaves)

PARENTS = [-1, 0, 0, 0, 1, 2, 3, 4, 5, 6, 7, 8, 9, 9, 9, 12, 13, 14, 16, 17, 18, 19, 20, 21]

# (child_lo, child_hi, parent_lo, parent_stride, parent_is_local)
COMPOSE_LEVELS = [
    (1, 4, 0, 0, True),
    (4, 7, 1, 1, False),
    (7, 10, 4, 1, False),
    (12, 15, 9, 0, False),
    (16, 18, 13, 1, False),
    (18, 20, 16, 1, False),
    (20, 22, 18, 1, False),
]
ROTATE_LEVELS = [
    (1, 4, 0, 0, True),
    (4, 7, 1, 1, False),
    (7, 10, 4, 1, False),
    (10, 13, 7, 1, False),
    (13, 15, 9, 0, False),
    (15, 18, 12, 1, False),
    (18, 20, 16, 1, False),
    (20, 22, 18, 1, False),
    (22, 24, 20, 1, False),
]

# stage-1 joint groups: group 0 unblocks compose L1 early
JGROUPS = [(0, 4), (4, 10), (10, 22)]

# output DMA chunks: (joint_lo, joint_hi, after_rotate_level_index)
OUT_CHUNKS = [(0, 10, 2), (10, 18, 5), (18, 24, 8)]

EPS = 1e-8
HALF_PI = float(np.pi / 2)

CO1_BASE = 0
CO2_BASE = 69
BASE_BASE = 138
EPS_COL = 210
HPI_COL = 211
PI_COL = 212
NCOEF = 213

DUPW = 5  # dup-vector slots per joint: [x, y, z, x, y]


def make_coef(offsets: np.ndarray) -> np.ndarray:
    offsets = np.asarray(offsets, dtype=np.float32)
    row = np.zeros((NCOEF,), dtype=np.float32)
    for c in range(1, 24):
        for i in range(3):
            row[CO1_BASE + (c - 1) * 3 + i] = 2.0 * offsets[c][(i + 2) % 3]
            row[CO2_BASE + (c - 1) * 3 + i] = 2.0 * offsets[c][(i + 1) % 3]
    base = np.zeros((24, 3), dtype=np.float32)
    base[0] = offsets[0]
    for j in range(1, 24):
        base[j] = base[PARENTS[j]] + offsets[j]
    row[BASE_BASE:BASE_BASE + 72] = base.reshape(-1)
    row[EPS_COL] = EPS
    row[HPI_COL] = HALF_PI
    row[PI_COL] = float(np.pi)
    return np.ascontiguousarray(np.broadcast_to(row, (P, NCOEF)))


def _ap(t_ap: bass.AP, off: int, dims) -> bass.AP:
    th = t_ap.tensor
    n = th.shape[1]
    return bass.AP(th, off, [[n, P]] + [[int(s), int(c)] for (s, c) in dims])


class Cfg:
    dt_q = mybir.dt.float32    # quaternion chain dtype
    dt_v = mybir.dt.float32    # V accumulation dtype
    coef16 = False             # keep a dt_q copy of COEF for 2x-mode muls
    final_on_gpsimd = False    # base-adds for joints < 18 on GPSIMD
    s1_on_gpsimd = False       # stage-1 k2/qv for groups 1,2 on GPSIMD
    dma_split = 2              # split input poses DMA along f
    split_out = True           # chunked output DMAs in (q, f) layout
    dup_on_act = False         # QD/TD dup copies on the Scalar engine
    bench_iters = 0            # wrap the whole pipeline in a For loop (timing)


def build_fk(tc: tile.TileContext, cfg: Cfg = None):
    cfg = cfg or Cfg()
    nc = tc.nc
    f32 = mybir.dt.float32
    dt_q, dt_v = cfg.dt_q, cfg.dt_v
    A = mybir.ActivationFunctionType
    OP = mybir.AluOpType

    poses = nc.dram_tensor("poses", [BC, J * 3], f32, kind="ExternalInput")
    trans = nc.dram_tensor("trans", [BC, 3], f32, kind="ExternalInput")
    coefd = nc.dram_tensor("coef", [P, NCOEF], f32, kind="ExternalInput")
    outd = nc.dram_tensor("positions", [P, 72 * F], f32, kind="ExternalOutput")

    pool = tc.alloc_tile_pool(name="main", bufs=1)

    def mk(name, n, dt):
        return pool.tile([P, n], dt, name=name)

    PR = mk("PR", F * 72, f32)
    TR = mk("TR", F * 3, f32)
    COEF = mk("COEF", NCOEF, f32)
    C16 = mk("C16", NCOEF, dt_q) if cfg.coef16 else None
    SQ = mk("SQ", 66 * F, f32)
    N2 = mk("N2", NJQ * F, f32)
    ANG = mk("ANG", NJQ * F, f32)
    INV = mk("INV", NJQ * F, f32)
    SH = mk("SH", NJQ * F, f32)
    K2 = mk("K2", NJQ * F, f32)
    LW = mk("LW", NJQ * F, dt_q)
    LD = mk("LD", NJQ * DUPW * F, dt_q)
    QW = mk("QW", NJQ * F, dt_q)
    QD = mk("QD", NJQ * DUPW * F, dt_q)
    V = mk("V", J * 3 * F, dt_v)
    OUT = mk("OUT", 72 * F, f32)          # dense (q, f): q*F + f

    T1 = mk("T1", 3 * 3 * F, dt_q)
    T2 = mk("T2", 3 * 3 * F, dt_q)
    T3 = mk("T3", 3 * 3 * F, dt_q)
    T4 = mk("T4", 3 * 3 * F, dt_q)
    V1 = mk("V1", 3 * 3 * F, dt_q)
    V2 = mk("V2", 3 * 3 * F, dt_q)
    PD = mk("PD", 3 * 3 * F, dt_q)
    PW = mk("PW", 3 * F, dt_q)
    TD = mk("TD", 3 * DUPW * F, dt_q)
    ZS = mk("ZS", 3 * 3 * F, dt_v)

    COEF_M = C16 if cfg.coef16 else COEF  # coef source for the 2x-mode muls

    import contextlib
    loop_ctx = tc.For_i(0, cfg.bench_iters, 1) if cfg.bench_iters else contextlib.nullcontext()
    with loop_ctx:
        _body(tc, cfg, nc, locals())
    pool.release()


def _body(tc, cfg, nc, env):
    f32 = mybir.dt.float32
    dt_q, dt_v = cfg.dt_q, cfg.dt_v
    A = mybir.ActivationFunctionType
    OP = mybir.AluOpType
    poses = env["poses"]; trans = env["trans"]; coefd = env["coefd"]; outd = env["outd"]
    PR = env["PR"]; TR = env["TR"]; COEF = env["COEF"]; C16 = env["C16"]
    SQ = env["SQ"]; N2 = env["N2"]; ANG = env["ANG"]; INV = env["INV"]
    SH = env["SH"]; K2 = env["K2"]; LW = env["LW"]; LD = env["LD"]
    QW = env["QW"]; QD = env["QD"]; V = env["V"]; OUT = env["OUT"]
    T1 = env["T1"]; T2 = env["T2"]; T3 = env["T3"]; T4 = env["T4"]
    V1 = env["V1"]; V2 = env["V2"]; PD = env["PD"]; PW = env["PW"]
    TD = env["TD"]; ZS = env["ZS"]; COEF_M = env["COEF_M"]

    # ---- DMA in: COEF/TR first (tiny, unblock ACT bias), poses split along
    # f across two DMA queues ----
    ns = cfg.dma_split
    fchunk = F // ns
    nc.sync.dma_start(COEF[:], coefd.ap())
    nc.sync.dma_start(TR[:], bass.AP(trans, 0, [[F * 3, P], [1, F * 3]]))
    dma_engs = [nc.sync, nc.scalar]
    for s in range(ns):
        dma_engs[s % len(dma_engs)].dma_start(
            _ap(PR, s * fchunk * 72, [(1, fchunk * 72)]),
            bass.AP(poses, s * fchunk * 72, [[F * 72, P], [1, fchunk * 72]]),
        )
    if cfg.coef16:
        nc.vector.tensor_copy(C16[:], COEF[:])

    # ---- stage 1: to-quat, per joint group x f-chunk ----
    def stage1(gi, jlo, jhi):
        njg = jhi - jlo
        # phase A (sqrt table set)
        for s in range(ns):
            f0, nf = s * fchunk, fchunk
            nc.scalar.activation(
                _ap(SQ, jlo * 3 * F + f0, [(F, 3 * njg), (1, nf)]),
                _ap(PR, f0 * 72 + jlo * 3, [(1, 3 * njg), (72, nf)]),
                A.Square, bias=_ap(COEF, EPS_COL, [(1, 1)]), scale=1.0,
            )
            nc.vector.tensor_tensor(
                _ap(N2, jlo * F + f0, [(F, njg), (1, nf)]),
                _ap(SQ, jlo * 3 * F + f0, [(3 * F, njg), (1, nf)]),
                _ap(SQ, jlo * 3 * F + F + f0, [(3 * F, njg), (1, nf)]),
                OP.add,
            )
            nc.vector.tensor_tensor(
                _ap(N2, jlo * F + f0, [(F, njg), (1, nf)]),
                _ap(N2, jlo * F + f0, [(F, njg), (1, nf)]),
                _ap(SQ, jlo * 3 * F + 2 * F + f0, [(3 * F, njg), (1, nf)]),
                OP.add,
            )
            nc.scalar.activation(
                _ap(ANG, jlo * F + f0, [(F, njg), (1, nf)]),
                _ap(N2, jlo * F + f0, [(F, njg), (1, nf)]), A.Sqrt)
            nc.vector.reciprocal_approx_fast(
                _ap(INV, jlo * F + f0, [(F, njg), (1, nf)]),
                _ap(ANG, jlo * F + f0, [(F, njg), (1, nf)]))
        # phase B (trig table set): sh = sin(pi - a/2); cos = sin(pi/2 - a/2)
        eng = nc.gpsimd if (cfg.s1_on_gpsimd and gi == 2) else nc.vector
        for s in range(ns):
            f0, nf = s * fchunk, fchunk
            nc.scalar.activation(
                _ap(SH, jlo * F + f0, [(F, njg), (1, nf)]),
                _ap(ANG, jlo * F + f0, [(F, njg), (1, nf)]), A.Sin, scale=-0.5,
                bias=_ap(COEF, PI_COL, [(1, 1)]))
            nc.scalar.activation(
                _ap(LW, jlo * F + f0, [(F, njg), (1, nf)]),
                _ap(ANG, jlo * F + f0, [(F, njg), (1, nf)]), A.Sin, scale=-0.5,
                bias=_ap(COEF, HPI_COL, [(1, 1)]),
            )
            eng.tensor_tensor(
                _ap(K2, jlo * F + f0, [(F, njg), (1, nf)]),
                _ap(SH, jlo * F + f0, [(F, njg), (1, nf)]),
                _ap(INV, jlo * F + f0, [(F, njg), (1, nf)]), OP.mult)
            eng.tensor_tensor(
                _ap(LD, jlo * DUPW * F + f0, [(DUPW * F, njg), (F, 3), (1, nf)]),
                _ap(K2, jlo * F + f0, [(F, njg), (0, 3), (1, nf)]),
                _ap(PR, f0 * 72 + jlo * 3, [(3, njg), (1, 3), (72, nf)]),
                OP.mult,
            )
        eng.tensor_copy(
            _ap(LD, jlo * DUPW * F + 3 * F, [(DUPW * F, njg), (1, 2 * F)]),
            _ap(LD, jlo * DUPW * F, [(DUPW * F, njg), (1, 2 * F)]),
        )

    stage1(0, *JGROUPS[0])
    # V[0] = TR rearranged (i, f)
    nc.vector.tensor_copy(
        _ap(V, 0, [(F, 3), (1, F)]),
        _ap(TR, 0, [(1, 3), (3, F)]),
    )
    stage1(1, *JGROUPS[1])
    stage1(2, *JGROUPS[2])

    def emit_final_add(jlo, jhi, eng):
        # OUT[3j+i, f] = V[3j+i, f] + base[3j+i]  (dense q-major)
        q0, nq = jlo * 3, (jhi - jlo) * 3
        eng.tensor_tensor(
            _ap(OUT, q0 * F, [(F, nq), (1, F)]),
            _ap(V, q0 * F, [(F, nq), (1, F)]),
            _ap(COEF, BASE_BASE + q0, [(1, nq), (0, F)]),
            OP.add,
        )

    fin_eng0 = nc.gpsimd if cfg.final_on_gpsimd else nc.vector
    emit_final_add(0, 1, fin_eng0)

    def qsrc(local):
        return (LW, LD) if local else (QW, QD)

    # ---- stage 2 + 3 interleaved per level would be natural, but compose
    # levels run ahead of rotate anyway; emit compose then rotate.
    for (clo, chi, plo, pstr, plocal) in COMPOSE_LEVELS:
        b = chi - clo
        w_, d_ = qsrc(plocal)
        pw = _ap(w_, plo * F, [(pstr * F, b), (0, 3), (1, F)])
        pwf = _ap(w_, plo * F, [(pstr * F, b), (1, F)])
        pv = _ap(d_, plo * DUPW * F, [(pstr * DUPW * F, b), (F, 3), (1, F)])
        pv1 = _ap(d_, plo * DUPW * F + F, [(pstr * DUPW * F, b), (F, 3), (1, F)])
        pv2 = _ap(d_, plo * DUPW * F + 2 * F, [(pstr * DUPW * F, b), (F, 3), (1, F)])
        cw = _ap(LW, clo * F, [(F, b), (0, 3), (1, F)])
        cwf = _ap(LW, clo * F, [(F, b), (1, F)])
        cv = _ap(LD, clo * DUPW * F, [(DUPW * F, b), (F, 3), (1, F)])
        cv1 = _ap(LD, clo * DUPW * F + F, [(DUPW * F, b), (F, 3), (1, F)])
        cv2 = _ap(LD, clo * DUPW * F + 2 * F, [(DUPW * F, b), (F, 3), (1, F)])

        def s3(t):
            return _ap(t, 0, [(3 * F, b), (F, 3), (1, F)])

        t1, t2, t3, t4, v1, v2, pd = map(s3, (T1, T2, T3, T4, V1, V2, PD))

        nc.vector.tensor_tensor(t1, pw, cv, OP.mult)
        nc.vector.tensor_tensor(t2, pv, cw, OP.mult)
        nc.vector.tensor_tensor(t3, pv1, cv2, OP.mult)
        nc.vector.tensor_tensor(t4, pv2, cv1, OP.mult)
        nc.vector.tensor_tensor(v1, t1, t2, OP.add)
        nc.vector.tensor_tensor(v2, t3, t4, OP.subtract)
        qdv = _ap(QD, clo * DUPW * F, [(DUPW * F, b), (F, 3), (1, F)])
        nc.vector.tensor_tensor(qdv, v1, v2, OP.add)
        if cfg.dup_on_act:
            nc.scalar.copy(
                _ap(QD, clo * DUPW * F + 3 * F, [(DUPW * F, b), (1, 2 * F)]),
                _ap(QD, clo * DUPW * F, [(DUPW * F, b), (1, 2 * F)]),
            )
        else:
            nc.vector.tensor_copy(
                _ap(QD, clo * DUPW * F + 3 * F, [(DUPW * F, b), (1, 2 * F)]),
                _ap(QD, clo * DUPW * F, [(DUPW * F, b), (1, 2 * F)]),
            )
        nc.vector.tensor_tensor(pd, pv, cv, OP.mult)
        pwc = _ap(PW, 0, [(F, b), (1, F)])
        nc.vector.tensor_tensor(pwc, pwf, cwf, OP.mult)
        qwc = _ap(QW, clo * F, [(F, b), (1, F)])
        nc.vector.tensor_tensor(qwc, pwc, _ap(PD, 0, [(3 * F, b), (1, F)]), OP.subtract)
        nc.vector.tensor_tensor(qwc, qwc, _ap(PD, F, [(3 * F, b), (1, F)]), OP.subtract)
        nc.vector.tensor_tensor(qwc, qwc, _ap(PD, 2 * F, [(3 * F, b), (1, F)]), OP.subtract)

    # ---- stage 3: rotate offsets, accumulate V; final adds + out DMA chunks ----
    chunk_after = {lvl: (jlo, jhi) for (jlo, jhi, lvl) in OUT_CHUNKS}
    for li, (clo, chi, plo, pstr, plocal) in enumerate(ROTATE_LEVELS):
        b = chi - clo
        w_, d_ = qsrc(plocal)
        pwk = _ap(w_, plo * F, [(pstr * F, b), (0, 3), (1, F)])
        pv1 = _ap(d_, plo * DUPW * F + F, [(pstr * DUPW * F, b), (F, 3), (1, F)])
        pv2 = _ap(d_, plo * DUPW * F + 2 * F, [(pstr * DUPW * F, b), (F, 3), (1, F)])
        co1 = _ap(COEF_M, CO1_BASE + (clo - 1) * 3, [(3, b), (1, 3), (0, F)])
        co2 = _ap(COEF_M, CO2_BASE + (clo - 1) * 3, [(3, b), (1, 3), (0, F)])

        def s3(t):
            return _ap(t, 0, [(3 * F, b), (F, 3), (1, F)])

        r1, r2, m1, m2, u, sv = map(s3, (T1, T2, T3, T4, V1, V2))
        z = _ap(ZS, 0, [(3 * F, b), (F, 3), (1, F)])

        nc.vector.tensor_tensor(r1, pv1, co1, OP.mult)
        nc.vector.tensor_tensor(r2, pv2, co2, OP.mult)
        td0 = _ap(TD, 0, [(DUPW * F, b), (F, 3), (1, F)])
        nc.vector.tensor_tensor(td0, r1, r2, OP.subtract)
        if cfg.dup_on_act:
            nc.scalar.copy(
                _ap(TD, 3 * F, [(DUPW * F, b), (1, 2 * F)]),
                _ap(TD, 0, [(DUPW * F, b), (1, 2 * F)]),
            )
        else:
            nc.vector.tensor_copy(
                _ap(TD, 3 * F, [(DUPW * F, b), (1, 2 * F)]),
                _ap(TD, 0, [(DUPW * F, b), (1, 2 * F)]),
            )
        td1 = _ap(TD, F, [(DUPW * F, b), (F, 3), (1, F)])
        td2 = _ap(TD, 2 * F, [(DUPW * F, b), (F, 3), (1, F)])
        nc.vector.tensor_tensor(m1, pv1, td2, OP.mult)
        nc.vector.tensor_tensor(m2, pv2, td1, OP.mult)
        nc.vector.tensor_tensor(u, m1, m2, OP.subtract)
        nc.vector.tensor_tensor(sv, pwk, td0, OP.mult)
        nc.vector.tensor_tensor(z, sv, u, OP.add)
        vc = _ap(V, clo * 3 * F, [(3 * F, b), (F, 3), (1, F)])
        vp = _ap(V, plo * 3 * F, [(pstr * 3 * F, b), (F, 3), (1, F)])
        nc.vector.tensor_tensor(vc, vp, z, OP.add)
        fin_eng = nc.gpsimd if (cfg.final_on_gpsimd and clo < 18) else nc.vector
        emit_final_add(clo, chi, fin_eng)
        if cfg.split_out and li in chunk_after:
            jlo, jhi = chunk_after[li]
            q0, nq = jlo * 3 * F, (jhi - jlo) * 3 * F
            nc.sync.dma_start(
                bass.AP(outd, q0, [[72 * F, P], [1, nq]]),
                _ap(OUT, q0, [(1, nq)]),
            )
    if not cfg.split_out:
        nc.sync.dma_start(bass.AP(outd, 0, [[72 * F, P], [1, 72 * F]]), OUT[:])


def build_program(dt_q=mybir.dt.float32, trn="TRN2", cfg: Cfg = None):
    if cfg is None:
        cfg = Cfg()
        cfg.dt_q = dt_q
    nc = bacc.Bacc(trn, target_bir_lowering=False, debug=False)
    with tile.TileContext(nc) as tc:
        build_fk(tc, cfg)
    nc.compile()
    return nc


def shard_inputs(inputs: dict, n_cores: int = 8):
    poses = np.asarray(inputs["poses"], dtype=np.float32).reshape(-1, J * 3)
    trans = np.asarray(inputs["trans"], dtype=np.float32).reshape(-1, 3)
    coef = make_coef(np.asarray(inputs["offsets"], dtype=np.float32))
    bc = poses.shape[0] // n_cores
    in_maps = []
    for c in range(n_cores):
        in_maps.append({
            "poses": np.ascontiguousarray(poses[c * bc:(c + 1) * bc]),
            "trans": np.ascontiguousarray(trans[c * bc:(c + 1) * bc]),
            "coef": coef,
        })
    return in_maps


def unshard_outputs(results) -> np.ndarray:
    # per-core output is [P, 72*F] in (partition, q, f) layout
    outs = []
    for r in results:
        o = r["positions"].reshape(P, 72, F).transpose(0, 2, 1)  # -> (p, f, q)
        outs.append(o.reshape(BC, J, 3))
    return np.concatenate(outs, axis=0)


# ======================== runtime entry point ========================

from concourse import bass_utils  # noqa: E402

N_CORES = 8
B = BC * N_CORES

LAST_EXEC_NS = None
_CACHED = {}


def _default_cfg():
    cfg = Cfg()
    cfg.dt_q = mybir.dt.float16
    cfg.dt_v = mybir.dt.float16
    cfg.coef16 = True
    cfg.final_on_gpsimd = True
    cfg.s1_on_gpsimd = True
    cfg.dup_on_act = True
    cfg.dma_split = 2
    cfg.split_out = True
    return cfg


def _get_program():
    if "nc" not in _CACHED:
        _CACHED["nc"] = build_program(cfg=_default_cfg())
    return _CACHED["nc"]


def kernel(offsets: np.ndarray, poses: np.ndarray, trans: np.ndarray) -> np.ndarray:
    global LAST_EXEC_NS
    nc = _get_program()
    in_maps = shard_inputs(
        {"offsets": offsets, "poses": poses, "trans": trans}, n_cores=N_CORES
    )
    res = bass_utils.run_bass_kernel_spmd(
        nc, in_maps, core_ids=list(range(N_CORES)),
    )
    LAST_EXEC_NS = res.exec_time_ns
    out = unshard_outputs(res.results)
    return np.ascontiguousarray(out.astype(np.float32))
